# revision 6
# baseline (speedup 1.0000x reference)
"""FFT-based linear convolution of two 2^23-point real signals on 8 trn2 NeuronCores.

Math: conv(a, x) = Im(ifft(fft(a + i*x)^2)) / 2, with the 2^24-point FFT done as a
3-factor (256^3) matmul FFT. Stage A (over n1) is computed r-sharded across cores,
one AllToAll reshards to k1-sharded for the middle row-FFTs (stages B, C), the
pointwise square happens in the digit-reversed domain, then the inverse stages
(C', B') run locally, a second AllToAll reshards back, and inverse stage A'
produces only the imaginary part of the first half of the time-domain signal.

Both AllToAlls are split into G pieces along the k1 dim so they overlap with the
middle row-FFT loop: middle group g starts once piece g has landed, and the
piece-g output collective flies while group g+1 computes. A k1 relabeling
(host-side permutation of the stage-A stationary columns, T1 rows and inverse
stage-A weight rows) makes each piece a contiguous 32-partition slab so pieces
live in separate DRAM tensors with precise Tile dependencies.
"""
import os
import numpy as np

os.environ.setdefault("JAX_PLATFORMS", "")
import jax

jax.config.update("jax_compilation_cache_dir", "/tmp/jax_neff_cache")
jax.config.update("jax_persistent_cache_min_entry_size_bytes", -1)
jax.config.update("jax_persistent_cache_min_compile_time_secs", 0)

import concourse.bass as bass
import concourse.tile as tile
from concourse import bacc, mybir
from concourse.bass_utils import run_bass_kernel_spmd

N = 8388608          # input length
M = 2 * N            # FFT size = 2^24
B = 256              # radix
R = B * B            # 65536
W = 8                # cores
RL = R // W          # 8192 columns of r per core
CH = 512             # free-dim chunk in stages A / A'
NCHUNK = RL // CH    # 16
G = 4                # A2A pieces (k1l split 32 -> G groups of 32/G)
GL = 32 // G         # k1l values per piece
F32 = mybir.dt.float32

# matmul operands use float32r (~13-bit mantissa, 4x the fp32 PE rate).
USE_F32R = True
MMD = mybir.dt.float32r if USE_F32R else F32


def _mm(ap):
    return ap


def build_nc(reps=1):
    nc = bacc.Bacc("TRN2", target_bir_lowering=False, debug=False, num_devices=W)

    a_in = nc.dram_tensor("a_c", [128, RL], MMD, kind="ExternalInput")
    x_in = nc.dram_tensor("x_c", [128, RL], MMD, kind="ExternalInput")
    t1r_in = nc.dram_tensor("t1r", [B, RL], F32, kind="ExternalInput")
    t1i_in = nc.dram_tensor("t1i", [B, RL], F32, kind="ExternalInput")
    pdr_in = nc.dram_tensor("pdr", [128, B], MMD, kind="ExternalInput")
    pdi_in = nc.dram_tensor("pdi", [128, B], MMD, kind="ExternalInput")
    pndi_in = nc.dram_tensor("pndi", [128, B], MMD, kind="ExternalInput")
    dr_in = nc.dram_tensor("dr", [B, B], MMD, kind="ExternalInput")
    di_in = nc.dram_tensor("di", [B, B], MMD, kind="ExternalInput")
    ndi_in = nc.dram_tensor("ndi", [B, B], MMD, kind="ExternalInput")
    t2r_in = nc.dram_tensor("t2r", [B, B], F32, kind="ExternalInput")
    t2i_in = nc.dram_tensor("t2i", [B, B], F32, kind="ExternalInput")
    aw1_in = nc.dram_tensor("aw1", [B, 128], MMD, kind="ExternalInput")
    aw2_in = nc.dram_tensor("aw2", [B, 128], MMD, kind="ExternalInput")
    y_out = nc.dram_tensor("y_c", [128, RL], F32, kind="ExternalOutput")

    rg = [list(range(W))]

    with tile.TileContext(nc) as tc:
        with tc.tile_pool(name="dram", bufs=1, space="DRAM") as dram, \
             tc.tile_pool(name="consts", bufs=1) as consts:
            cc1i = [dram.tile([W, GL, 2, RL], MMD, name=f"cc1i_{g}", tag=f"cc1i_{g}")
                    for g in range(G)]
            cc1o = [dram.tile([W, GL, 2, RL], MMD, name=f"cc1o_{g}", tag=f"cc1o_{g}")
                    for g in range(G)]
            cc2i = [dram.tile([W, GL, 2, 32, B], F32, name=f"cc2i_{g}", tag=f"cc2i_{g}")
                    for g in range(G)]
            cc2o = [dram.tile([W, GL, 2, 32, B], F32, name=f"cc2o_{g}", tag=f"cc2o_{g}")
                    for g in range(G)]

            # ---- constant tables in SBUF ----
            # phase-A stationary (cols k1-permuted), [128 n1, 256 k1p]
            pa_st = []
            for src in (pdr_in, pdi_in, pndi_in):
                t = consts.tile([128, B], MMD, name=f"c_{src.name}", tag=f"c_{src.name}")
                nc.sync.dma_start(t[:], src[:, :])
                pa_st.append(t)
            pdr_t, pdi_t, pndi_t = pa_st
            # middle D row-halves (128, 256); col-slices give 128x128 blocks.
            dr_row, di_row, ndi_row, t2r_row, t2i_row = [], [], [], [], []
            for p in range(2):
                for lst, src, dt_ in ((dr_row, dr_in, MMD), (di_row, di_in, MMD), (ndi_row, ndi_in, MMD),
                                 (t2r_row, t2r_in, F32), (t2i_row, t2i_in, F32)):
                    t = consts.tile([128, B], dt_, name=f"c_{src.name}_{p}", tag=f"c_{src.name}_{p}")
                    nc.sync.dma_start(t[:], src[128 * p:128 * (p + 1), :])
                    lst.append(t)
            aw1_blk, aw2_blk = [], []
            for p in range(2):
                for lst, src in ((aw1_blk, aw1_in), (aw2_blk, aw2_in)):
                    t = consts.tile([128, 128], MMD, name=f"c_{src.name}_{p}", tag=f"c_{src.name}_{p}")
                    nc.sync.dma_start(t[:], src[128 * p:128 * (p + 1), :])
                    lst.append(t)

            for _rep in range(reps):
                # ================= Phase A: stage A + T1 twiddle =================
                with tc.tile_pool(name="a_io", bufs=1) as a_io, \
                     tc.tile_pool(name="a_t1", bufs=8) as a_t1, \
                     tc.tile_pool(name="a_tmp", bufs=16) as a_tmp, \
                     tc.tile_pool(name="a_out", bufs=6) as a_outp, \
                     tc.tile_pool(name="a_ps", bufs=4, space="PSUM") as a_ps:
                    a_full = a_io.tile([128, RL], MMD)
                    nc.sync.dma_start(a_full[:], a_in[:, :])
                    x_full = a_io.tile([128, RL], MMD)
                    nc.sync.dma_start(x_full[:], x_in[:, :])

                    for c in range(NCHUNK):
                        a_sl = a_full[:, c * CH:(c + 1) * CH]
                        x_sl = x_full[:, c * CH:(c + 1) * CH]
                        for h in range(2):
                            hs = slice(128 * h, 128 * (h + 1))
                            ps_r = a_ps.tile([128, CH], F32, tag="ps")
                            ps_i = a_ps.tile([128, CH], F32, tag="ps")
                            nc.tensor.matmul(ps_r[:], _mm(pdr_t[:, hs]), _mm(a_sl),
                                             start=True, stop=False)
                            nc.tensor.matmul(ps_i[:], _mm(pdr_t[:, hs]), _mm(x_sl),
                                             start=True, stop=False)
                            nc.tensor.matmul(ps_r[:], _mm(pndi_t[:, hs]), _mm(x_sl),
                                             start=False, stop=True)
                            nc.tensor.matmul(ps_i[:], _mm(pdi_t[:, hs]), _mm(a_sl),
                                             start=False, stop=True)

                            t1r_t = a_t1.tile([128, CH], F32, tag="t1")
                            nc.sync.dma_start(t1r_t[:], t1r_in[hs, c * CH:(c + 1) * CH])
                            t1i_t = a_t1.tile([128, CH], F32, tag="t1")
                            nc.sync.dma_start(t1i_t[:], t1i_in[hs, c * CH:(c + 1) * CH])

                            # Y' = (ps_r + i ps_i) * (t1r + i t1i), packed [Re | Im]
                            out_t = a_outp.tile([128, 2 * CH], MMD, tag="aout")
                            m1 = a_tmp.tile([128, CH], F32, tag="tmp")
                            m2 = a_tmp.tile([128, CH], F32, tag="tmp")
                            m3 = a_tmp.tile([128, CH], F32, tag="tmp")
                            m4 = a_tmp.tile([128, CH], F32, tag="tmp")
                            nc.vector.tensor_mul(m1[:], ps_r[:], t1r_t[:])
                            nc.vector.tensor_mul(m2[:], ps_i[:], t1i_t[:])
                            nc.vector.tensor_mul(m3[:], ps_r[:], t1i_t[:])
                            nc.vector.tensor_mul(m4[:], ps_i[:], t1r_t[:])
                            nc.gpsimd.tensor_sub(out_t[:, 0:CH], m1[:], m2[:])
                            nc.gpsimd.tensor_add(out_t[:, CH:2 * CH], m3[:], m4[:])

                            # store piece slabs: partitions [32g,32g+32) hold
                            # k1 = 128h + 32j + 8g + l (j-major, l-minor)
                            for g in range(G):
                                nc.sync.dma_start(
                                    cc1i[g][4 * h:4 * (h + 1), :, :, c * CH:(c + 1) * CH],
                                    out_t[32 * g:32 * (g + 1), :])

                for g in range(G):
                    nc.gpsimd.collective_compute(
                        "AllToAll", mybir.AluOpType.bypass, replica_groups=rg,
                        ins=[cc1i[g].opt()], outs=[cc1o[g].opt()])

                # ================= Middle: per-k1 row FFT + square ================
                with tc.tile_pool(name="m_in", bufs=16) as m_in, \
                     tc.tile_pool(name="m_sb", bufs=16) as m_sb, \
                     tc.tile_pool(name="m_out", bufs=12) as m_out, \
                     tc.tile_pool(name="m_ps", bufs=8, space="PSUM") as m_ps:
                  for g in range(G):
                    for l in range(GL):
                        # load Y[k1] as (n2, n3), split in n2-halves, planes
                        y_t = []  # [n2h][plane]
                        for n2h in range(2):
                            row = []
                            for pl in range(2):
                                t = m_in.tile([128, B], MMD, tag="yin")
                                nc.sync.dma_start(
                                    t[:], cc1o[g][4 * n2h:4 * (n2h + 1), l, pl, :])
                                row.append(t)
                            y_t.append(row)

                        # stage B (data as weights): Z^T (n3, k2) in PSUM
                        zt_ps = []  # [n3h][plane]
                        for n3h in range(2):
                            ns = slice(128 * n3h, 128 * (n3h + 1))
                            zr = m_ps.tile([128, B], F32, tag="mps")
                            zi = m_ps.tile([128, B], F32, tag="mps")
                            for n2h in range(2):
                                st = n2h == 0
                                sp = n2h == 1
                                nc.tensor.matmul(zr[:], _mm(y_t[n2h][0][:, ns]), _mm(dr_row[n2h][:]),
                                                 start=st, stop=False, skip_group_check=True)
                                nc.tensor.matmul(zi[:], _mm(y_t[n2h][0][:, ns]), _mm(di_row[n2h][:]),
                                                 start=st, stop=False, skip_group_check=True)
                                nc.tensor.matmul(zr[:], _mm(y_t[n2h][1][:, ns]), _mm(ndi_row[n2h][:]),
                                                 start=False, stop=sp, skip_group_check=True)
                                nc.tensor.matmul(zi[:], _mm(y_t[n2h][1][:, ns]), _mm(dr_row[n2h][:]),
                                                 start=False, stop=sp, skip_group_check=True)
                            zt_ps.append((zr, zi))

                        # T2 twiddle (Z^T layout: mult by T2^T rows) -> SBUF
                        zt_sb = []
                        for n3h in range(2):
                            zr, zi = zt_ps[n3h]
                            or_ = m_sb.tile([128, B], MMD, tag="zt")
                            oi_ = m_sb.tile([128, B], MMD, tag="zt")
                            p1 = m_sb.tile([128, B], F32, tag="mtmp")
                            p2 = m_sb.tile([128, B], F32, tag="mtmp")
                            p3 = m_sb.tile([128, B], F32, tag="mtmp")
                            p4 = m_sb.tile([128, B], F32, tag="mtmp")
                            nc.vector.tensor_mul(p1[:], zr[:], t2r_row[n3h][:])
                            nc.vector.tensor_mul(p2[:], zi[:], t2i_row[n3h][:])
                            nc.vector.tensor_mul(p3[:], zr[:], t2i_row[n3h][:])
                            nc.vector.tensor_mul(p4[:], zi[:], t2r_row[n3h][:])
                            nc.gpsimd.tensor_sub(or_[:], p1[:], p2[:])
                            nc.gpsimd.tensor_add(oi_[:], p3[:], p4[:])
                            zt_sb.append((or_, oi_))

                        # stage C (DFT stationary): U^T (k3, k2) in PSUM
                        ut_ps = []
                        for k3h in range(2):
                            ks = slice(128 * k3h, 128 * (k3h + 1))
                            ur = m_ps.tile([128, B], F32, tag="mps")
                            ui = m_ps.tile([128, B], F32, tag="mps")
                            for n3h in range(2):
                                st = n3h == 0
                                sp = n3h == 1
                                nc.tensor.matmul(ur[:], _mm(dr_row[n3h][:, ks]), _mm(zt_sb[n3h][0][:]),
                                                 start=st, stop=False, skip_group_check=True)
                                nc.tensor.matmul(ui[:], _mm(di_row[n3h][:, ks]), _mm(zt_sb[n3h][0][:]),
                                                 start=st, stop=False, skip_group_check=True)
                                nc.tensor.matmul(ur[:], _mm(ndi_row[n3h][:, ks]), _mm(zt_sb[n3h][1][:]),
                                                 start=False, stop=sp, skip_group_check=True)
                                nc.tensor.matmul(ui[:], _mm(dr_row[n3h][:, ks]), _mm(zt_sb[n3h][1][:]),
                                                 start=False, stop=sp, skip_group_check=True)
                            ut_ps.append((ur, ui))

                        # square: S = U^2 (k3, k2) -> SBUF
                        s_sb = []
                        for k3h in range(2):
                            ur, ui = ut_ps[k3h]
                            sr = m_sb.tile([128, B], MMD, tag="ssb")
                            si = m_sb.tile([128, B], MMD, tag="ssb")
                            uc = m_sb.tile([128, B], F32, tag="mtmp")
                            q1 = m_sb.tile([128, B], F32, tag="mtmp")
                            q2 = m_sb.tile([128, B], F32, tag="mtmp")
                            # only one PSUM operand allowed per vector op: stage ur in SBUF
                            nc.scalar.copy(uc[:], ur[:])
                            nc.vector.tensor_add(q1[:], uc[:], ui[:])
                            nc.vector.tensor_sub(q2[:], uc[:], ui[:])
                            nc.vector.scalar_tensor_tensor(
                                si[:], uc[:], 2.0, ui[:],
                                mybir.AluOpType.mult, mybir.AluOpType.mult)
                            nc.gpsimd.tensor_mul(sr[:], q1[:], q2[:])
                            s_sb.append((sr, si))

                        # stage C' (data as weights): Z2 (k2, n3) in PSUM
                        z2_ps = []
                        for k2h in range(2):
                            ks = slice(128 * k2h, 128 * (k2h + 1))
                            zr = m_ps.tile([128, B], F32, tag="mps")
                            zi = m_ps.tile([128, B], F32, tag="mps")
                            for k3h in range(2):
                                st = k3h == 0
                                sp = k3h == 1
                                nc.tensor.matmul(zr[:], _mm(s_sb[k3h][0][:, ks]), _mm(dr_row[k3h][:]),
                                                 start=st, stop=False, skip_group_check=True)
                                nc.tensor.matmul(zi[:], _mm(s_sb[k3h][0][:, ks]), _mm(ndi_row[k3h][:]),
                                                 start=st, stop=False, skip_group_check=True)
                                nc.tensor.matmul(zr[:], _mm(s_sb[k3h][1][:, ks]), _mm(di_row[k3h][:]),
                                                 start=False, stop=sp, skip_group_check=True)
                                nc.tensor.matmul(zi[:], _mm(s_sb[k3h][1][:, ks]), _mm(dr_row[k3h][:]),
                                                 start=False, stop=sp, skip_group_check=True)
                            z2_ps.append((zr, zi))

                        # conj(T2) twiddle (natural (k2, n3) layout) -> SBUF
                        y2_sb = []
                        for k2h in range(2):
                            zr, zi = z2_ps[k2h]
                            or_ = m_sb.tile([128, B], MMD, tag="y2")
                            oi_ = m_sb.tile([128, B], MMD, tag="y2")
                            p1 = m_sb.tile([128, B], F32, tag="mtmp")
                            p2 = m_sb.tile([128, B], F32, tag="mtmp")
                            p3 = m_sb.tile([128, B], F32, tag="mtmp")
                            p4 = m_sb.tile([128, B], F32, tag="mtmp")
                            nc.vector.tensor_mul(p1[:], zr[:], t2r_row[k2h][:])
                            nc.vector.tensor_mul(p2[:], zi[:], t2i_row[k2h][:])
                            nc.vector.tensor_mul(p3[:], zi[:], t2r_row[k2h][:])
                            nc.vector.tensor_mul(p4[:], zr[:], t2i_row[k2h][:])
                            nc.gpsimd.tensor_add(or_[:], p1[:], p2[:])
                            nc.gpsimd.tensor_sub(oi_[:], p3[:], p4[:])
                            y2_sb.append((or_, oi_))

                        # stage B' (DFT stationary, conj D): Y' (n2, n3) in PSUM
                        for n2h in range(2):
                            ns = slice(128 * n2h, 128 * (n2h + 1))
                            yr = m_ps.tile([128, B], F32, tag="mps")
                            yi = m_ps.tile([128, B], F32, tag="mps")
                            for k2h in range(2):
                                st = k2h == 0
                                sp = k2h == 1
                                nc.tensor.matmul(yr[:], _mm(dr_row[k2h][:, ns]), _mm(y2_sb[k2h][0][:]),
                                                 start=st, stop=False, skip_group_check=True)
                                nc.tensor.matmul(yi[:], _mm(dr_row[k2h][:, ns]), _mm(y2_sb[k2h][1][:]),
                                                 start=st, stop=False, skip_group_check=True)
                                nc.tensor.matmul(yr[:], _mm(di_row[k2h][:, ns]), _mm(y2_sb[k2h][1][:]),
                                                 start=False, stop=sp, skip_group_check=True)
                                nc.tensor.matmul(yi[:], _mm(ndi_row[k2h][:, ns]), _mm(y2_sb[k2h][0][:]),
                                                 start=False, stop=sp, skip_group_check=True)
                            # copy to SBUF and store to A2A-2 input
                            for pl, ps in ((0, yr), (1, yi)):
                                o = m_out.tile([128, B], F32, tag="mout")
                                nc.scalar.copy(o[:], ps[:])
                                nc.sync.dma_start(
                                    cc2i[g][4 * n2h:4 * (n2h + 1), l, pl, :, :], o[:])

                    # piece-g output collective flies while group g+1 computes
                    nc.gpsimd.collective_compute(
                        "AllToAll", mybir.AluOpType.bypass, replica_groups=rg,
                        ins=[cc2i[g].opt()], outs=[cc2o[g].opt()])

                # ============ Phase A': conj(T1), inverse stage A (Im only) ============
                with tc.tile_pool(name="f_in", bufs=16) as f_in, \
                     tc.tile_pool(name="f_t1", bufs=8) as f_t1, \
                     tc.tile_pool(name="f_tmp", bufs=16) as f_tmp, \
                     tc.tile_pool(name="f_out", bufs=6) as f_outp, \
                     tc.tile_pool(name="f_ps", bufs=4, space="PSUM") as f_ps:
                    for c in range(NCHUNK):
                        ps_o = f_ps.tile([128, CH], F32, tag="fps")
                        for h in range(2):
                            hs = slice(128 * h, 128 * (h + 1))
                            pr = f_in.tile([128, CH], F32, tag="pin")
                            pi = f_in.tile([128, CH], F32, tag="pin")
                            for g in range(G):
                                nc.sync.dma_start(
                                    pr[32 * g:32 * (g + 1), :],
                                    cc2o[g][4 * h:4 * (h + 1), :, 0, 2 * c:2 * (c + 1), :])
                                nc.sync.dma_start(
                                    pi[32 * g:32 * (g + 1), :],
                                    cc2o[g][4 * h:4 * (h + 1), :, 1, 2 * c:2 * (c + 1), :])
                            t1r_t = f_t1.tile([128, CH], F32, tag="ft1")
                            nc.sync.dma_start(t1r_t[:], t1r_in[hs, c * CH:(c + 1) * CH])
                            t1i_t = f_t1.tile([128, CH], F32, tag="ft1")
                            nc.sync.dma_start(t1i_t[:], t1i_in[hs, c * CH:(c + 1) * CH])

                            # Yf = P * conj(T1)
                            yfr = f_tmp.tile([128, CH], MMD, tag="yf")
                            yfi = f_tmp.tile([128, CH], MMD, tag="yf")
                            p1 = f_tmp.tile([128, CH], F32, tag="ftmp")
                            p2 = f_tmp.tile([128, CH], F32, tag="ftmp")
                            p3 = f_tmp.tile([128, CH], F32, tag="ftmp")
                            p4 = f_tmp.tile([128, CH], F32, tag="ftmp")
                            nc.vector.tensor_mul(p1[:], pr[:], t1r_t[:])
                            nc.gpsimd.tensor_mul(p2[:], pi[:], t1i_t[:])
                            nc.vector.tensor_mul(p3[:], pi[:], t1r_t[:])
                            nc.gpsimd.tensor_mul(p4[:], pr[:], t1i_t[:])
                            nc.vector.tensor_add(yfr[:], p1[:], p2[:])
                            nc.vector.tensor_sub(yfi[:], p3[:], p4[:])

                            st = h == 0
                            sp = h == 1
                            nc.tensor.matmul(ps_o[:], _mm(aw1_blk[h][:]), _mm(yfi[:]),
                                             start=st, stop=False, skip_group_check=True)
                            nc.tensor.matmul(ps_o[:], _mm(aw2_blk[h][:]), _mm(yfr[:]),
                                             start=False, stop=sp, skip_group_check=True)

                        o = f_outp.tile([128, CH], F32, tag="fout")
                        nc.scalar.copy(o[:], ps_o[:])
                        nc.sync.dma_start(y_out[:, c * CH:(c + 1) * CH], o[:])

    nc.compile()
    return nc


_NC = None
_TABLES = None


def _tables():
    global _TABLES
    if _TABLES is None:
        k = np.arange(B)
        D = np.exp(-2j * np.pi * np.outer(k, k) / B)
        T2 = np.exp(-2j * np.pi * np.outer(k, k) / R)
        s = 1.0 / (2.0 * M)
        dr = np.ascontiguousarray(D.real.astype(np.float32))
        di = np.ascontiguousarray(D.imag.astype(np.float32))
        # k1 relabeling: partition p (within an h-half) holds k1loc(p) where
        # p = 32g + 8j + l  ->  k1loc = 32j + 8g + l
        p = np.arange(128)
        k1loc = 32 * ((p % 32) // 8) + 8 * (p // 32) + (p % 8)
        perm = np.concatenate([k1loc, 128 + k1loc])  # rows (h*128+p) -> k1
        t1s = []
        for c in range(W):
            r = np.arange(c * RL, (c + 1) * RL)
            T1 = np.exp(-2j * np.pi * np.outer(k, r) / M)
            T1p = T1[perm, :]
            t1s.append((np.ascontiguousarray(T1p.real.astype(np.float32)),
                        np.ascontiguousarray(T1p.imag.astype(np.float32))))
        pd = D[:128, :][:, perm]
        aw1 = (s * D.real[:, :128])[perm, :]
        aw2 = (-s * D.imag[:, :128])[perm, :]
        _TABLES = dict(
            dr=dr, di=di, ndi=np.ascontiguousarray(-di),
            pdr=np.ascontiguousarray(pd.real.astype(np.float32)),
            pdi=np.ascontiguousarray(pd.imag.astype(np.float32)),
            pndi=np.ascontiguousarray(-pd.imag.astype(np.float32)),
            t2r=np.ascontiguousarray(T2.real.astype(np.float32)),
            t2i=np.ascontiguousarray(T2.imag.astype(np.float32)),
            aw1=np.ascontiguousarray(aw1.astype(np.float32)),
            aw2=np.ascontiguousarray(aw2.astype(np.float32)),
            t1s=t1s,
        )
    return _TABLES


def _in_maps(a, x):
    tb = _tables()
    a3 = a.reshape(128, W, RL)
    x3 = x.reshape(128, W, RL)
    in_maps = []
    for c in range(W):
        in_maps.append(dict(
            a_c=np.ascontiguousarray(a3[:, c, :]),
            x_c=np.ascontiguousarray(x3[:, c, :]),
            t1r=tb["t1s"][c][0], t1i=tb["t1s"][c][1],
            pdr=tb["pdr"], pdi=tb["pdi"], pndi=tb["pndi"],
            dr=tb["dr"], di=tb["di"], ndi=tb["ndi"],
            t2r=tb["t2r"], t2i=tb["t2i"],
            aw1=tb["aw1"], aw2=tb["aw2"],
        ))
    return in_maps


def kernel(a, x, _want_trace=False, **_unused):
    global _NC
    a = np.asarray(a, dtype=np.float32)
    x = np.asarray(x, dtype=np.float32)
    if _NC is None:
        _NC = build_nc()
    res = run_bass_kernel_spmd(_NC, _in_maps(a, x), core_ids=list(range(W)),
                               trace=_want_trace)
    full = np.empty((128, R), dtype=np.float32)
    for c in range(W):
        full[:, c * RL:(c + 1) * RL] = res.results[c]["y_c"]
    out = full.reshape(-1)
    if _want_trace:
        return out, res
    return out


# revision 16
# speedup vs baseline: 79.7122x; 79.7122x over previous
"""FFT-based linear convolution of two 2^23-point real signals on 8 trn2 NeuronCores.

Math: conv(a, x) = Im(ifft(fft(a + i*x)^2)) / 2, with the 2^24-point FFT done as a
3-factor (256^3) matmul FFT. Stage A (over n1) is computed r-sharded across cores,
one AllToAll reshards to k1-sharded for the middle row-FFTs (stages B, C), the
pointwise square happens in the digit-reversed domain, then the inverse stages
(C', B') run locally, a second AllToAll reshards back, and inverse stage A'
produces only the imaginary part of the first half of the time-domain signal.

Both AllToAlls are split into G pieces along the k1 dim so they overlap with the
middle row-FFT loop: middle group g starts once piece g has landed, and the
piece-g output collective flies while group g+1 computes. A k1 relabeling
(host-side permutation of the stage-A stationary columns, T1 rows and inverse
stage-A weight rows) makes each piece a contiguous 32-partition slab so pieces
live in separate DRAM tensors with precise Tile dependencies.
"""
import os
import numpy as np

os.environ.setdefault("JAX_PLATFORMS", "")
import jax

jax.config.update("jax_compilation_cache_dir", "/tmp/jax_neff_cache")
jax.config.update("jax_persistent_cache_min_entry_size_bytes", -1)
jax.config.update("jax_persistent_cache_min_compile_time_secs", 0)

import concourse.bass as bass
import concourse.tile as tile
from concourse import bacc, mybir
from concourse.bass_utils import run_bass_kernel_spmd

N = 8388608          # input length
M = 2 * N            # FFT size = 2^24
B = 256              # radix
R = B * B            # 65536
W = 8                # cores
RL = R // W          # 8192 columns of r per core
CH = 512             # free-dim chunk in stages A / A'
NCHUNK = RL // CH    # 16
G = 4                # A2A pieces (k1l split 32 -> G groups of 32/G)
GL = 32 // G         # k1l values per piece
F32 = mybir.dt.float32

# matmul operands use float32r (~13-bit mantissa, 4x the fp32 PE rate).
USE_F32R = True
MMD = mybir.dt.float32r if USE_F32R else F32
# A2A payloads (cc1/cc2) ride in bf16: halves HBM + link traffic for both
# collectives; loads are cast back to f32r/f32 before any matmul.
PAY = mybir.dt.bfloat16


def _mm(ap):
    return ap


def build_nc(reps=1, phases="AMF"):
    # phases: experiment-only knob ("A"=stage A, "M"=middle, "F"=stage A';
    # collectives always emitted). kernel() always uses the full "AMF".
    nc = bacc.Bacc("TRN2", target_bir_lowering=False, debug=False, num_devices=W)

    a_in = nc.dram_tensor("a_c", [128, RL], MMD, kind="ExternalInput")
    x_in = nc.dram_tensor("x_c", [128, RL], MMD, kind="ExternalInput")
    t1r_in = nc.dram_tensor("t1r", [B, RL], F32, kind="ExternalInput")
    t1i_in = nc.dram_tensor("t1i", [B, RL], F32, kind="ExternalInput")
    pdr_in = nc.dram_tensor("pdr", [128, B], MMD, kind="ExternalInput")
    pdi_in = nc.dram_tensor("pdi", [128, B], MMD, kind="ExternalInput")
    pndi_in = nc.dram_tensor("pndi", [128, B], MMD, kind="ExternalInput")
    dr_in = nc.dram_tensor("dr", [B, B], MMD, kind="ExternalInput")
    di_in = nc.dram_tensor("di", [B, B], MMD, kind="ExternalInput")
    ndi_in = nc.dram_tensor("ndi", [B, B], MMD, kind="ExternalInput")
    t2r_in = nc.dram_tensor("t2r", [B, B], F32, kind="ExternalInput")
    t2i_in = nc.dram_tensor("t2i", [B, B], F32, kind="ExternalInput")
    aw1_in = nc.dram_tensor("aw1", [B, 128], MMD, kind="ExternalInput")
    aw2_in = nc.dram_tensor("aw2", [B, 128], MMD, kind="ExternalInput")
    y_out = nc.dram_tensor("y_c", [128, RL], F32, kind="ExternalOutput")

    rg = [list(range(W))]

    with tile.TileContext(nc) as tc:
        with tc.tile_pool(name="dram", bufs=1, space="DRAM") as dram, \
             tc.tile_pool(name="consts", bufs=1) as consts:
            cc1i = [dram.tile([W, GL, 2, RL], PAY, name=f"cc1i_{g}", tag=f"cc1i_{g}")
                    for g in range(G)]
            cc1o = [dram.tile([W, GL, 2, RL], PAY, name=f"cc1o_{g}", tag=f"cc1o_{g}")
                    for g in range(G)]
            cc2i = [dram.tile([W, GL, 2, 32, B], PAY, name=f"cc2i_{g}", tag=f"cc2i_{g}")
                    for g in range(G)]
            cc2o = [dram.tile([W, GL, 2, 32, B], PAY, name=f"cc2o_{g}", tag=f"cc2o_{g}")
                    for g in range(G)]

            # ---- constant tables in SBUF ----
            # phase-A stationary (cols k1-permuted), [128 n1, 256 k1p]
            pa_st = []
            for src in (pdr_in, pdi_in, pndi_in):
                t = consts.tile([128, B], MMD, name=f"c_{src.name}", tag=f"c_{src.name}")
                nc.sync.dma_start(t[:], src[:, :])
                pa_st.append(t)
            pdr_t, pdi_t, pndi_t = pa_st
            # middle D row-halves (128, 256); col-slices give 128x128 blocks.
            dr_row, di_row, ndi_row, t2r_row, t2i_row = [], [], [], [], []
            for p in range(2):
                for lst, src, dt_ in ((dr_row, dr_in, MMD), (di_row, di_in, MMD), (ndi_row, ndi_in, MMD),
                                 (t2r_row, t2r_in, F32), (t2i_row, t2i_in, F32)):
                    t = consts.tile([128, B], dt_, name=f"c_{src.name}_{p}", tag=f"c_{src.name}_{p}")
                    nc.sync.dma_start(t[:], src[128 * p:128 * (p + 1), :])
                    lst.append(t)
            aw1_blk, aw2_blk = [], []
            for p in range(2):
                for lst, src in ((aw1_blk, aw1_in), (aw2_blk, aw2_in)):
                    t = consts.tile([128, 128], MMD, name=f"c_{src.name}_{p}", tag=f"c_{src.name}_{p}")
                    nc.sync.dma_start(t[:], src[128 * p:128 * (p + 1), :])
                    lst.append(t)

            for _rep in range(reps):
                # ================= Phase A: stage A + T1 twiddle =================
                if "A" in phases:
                 with tc.tile_pool(name="a_io", bufs=1) as a_io, \
                     tc.tile_pool(name="a_t1", bufs=8) as a_t1, \
                     tc.tile_pool(name="a_tmp", bufs=16) as a_tmp, \
                     tc.tile_pool(name="a_out", bufs=6) as a_outp, \
                     tc.tile_pool(name="a_ps", bufs=4, space="PSUM") as a_ps:
                    a_full = a_io.tile([128, RL], MMD)
                    nc.sync.dma_start(a_full[:], a_in[:, :])
                    x_full = a_io.tile([128, RL], MMD)
                    nc.sync.dma_start(x_full[:], x_in[:, :])

                    for c in range(NCHUNK):
                        a_sl = a_full[:, c * CH:(c + 1) * CH]
                        x_sl = x_full[:, c * CH:(c + 1) * CH]
                        for h in range(2):
                            hs = slice(128 * h, 128 * (h + 1))
                            ps_r = a_ps.tile([128, CH], F32, tag="ps")
                            ps_i = a_ps.tile([128, CH], F32, tag="ps")
                            nc.tensor.matmul(ps_r[:], _mm(pdr_t[:, hs]), _mm(a_sl),
                                             start=True, stop=False)
                            nc.tensor.matmul(ps_i[:], _mm(pdr_t[:, hs]), _mm(x_sl),
                                             start=True, stop=False)
                            nc.tensor.matmul(ps_r[:], _mm(pndi_t[:, hs]), _mm(x_sl),
                                             start=False, stop=True)
                            nc.tensor.matmul(ps_i[:], _mm(pdi_t[:, hs]), _mm(a_sl),
                                             start=False, stop=True)

                            t1r_t = a_t1.tile([128, CH], F32, tag="t1")
                            nc.sync.dma_start(t1r_t[:], t1r_in[hs, c * CH:(c + 1) * CH])
                            t1i_t = a_t1.tile([128, CH], F32, tag="t1")
                            nc.sync.dma_start(t1i_t[:], t1i_in[hs, c * CH:(c + 1) * CH])

                            # Y' = (ps_r + i ps_i) * (t1r + i t1i), packed [Re | Im]
                            out_t = a_outp.tile([128, 2 * CH], PAY, tag="aout")
                            m1 = a_tmp.tile([128, CH], F32, tag="tmp")
                            m2 = a_tmp.tile([128, CH], F32, tag="tmp")
                            m3 = a_tmp.tile([128, CH], F32, tag="tmp")
                            m4 = a_tmp.tile([128, CH], F32, tag="tmp")
                            nc.vector.tensor_mul(m1[:], ps_r[:], t1r_t[:])
                            nc.vector.tensor_mul(m2[:], ps_i[:], t1i_t[:])
                            nc.vector.tensor_mul(m3[:], ps_r[:], t1i_t[:])
                            nc.vector.tensor_mul(m4[:], ps_i[:], t1r_t[:])
                            nc.gpsimd.tensor_sub(out_t[:, 0:CH], m1[:], m2[:])
                            nc.gpsimd.tensor_add(out_t[:, CH:2 * CH], m3[:], m4[:])

                            # store piece slabs: partitions [32g,32g+32) hold
                            # k1 = 128h + 32j + 8g + l (j-major, l-minor)
                            for g in range(G):
                                nc.sync.dma_start(
                                    cc1i[g][4 * h:4 * (h + 1), :, :, c * CH:(c + 1) * CH],
                                    out_t[32 * g:32 * (g + 1), :])

                for g in range(G):
                    nc.gpsimd.collective_compute(
                        "AllToAll", mybir.AluOpType.bypass, replica_groups=rg,
                        ins=[cc1i[g].opt()], outs=[cc1o[g].opt()])

                # ================= Middle: per-k1 row FFT + square ================
                if "M" in phases:
                 with tc.tile_pool(name="m_in", bufs=16) as m_in, \
                     tc.tile_pool(name="m_sb", bufs=16) as m_sb, \
                     tc.tile_pool(name="m_out", bufs=12) as m_out, \
                     tc.tile_pool(name="m_ps", bufs=8, space="PSUM") as m_ps:
                  for g in range(G):
                    for l in range(GL):
                        # load Y[k1] as (n2, n3), split in n2-halves, planes;
                        # cast bf16 payload -> f32r for the stage-B stationaries
                        y_t = []  # [n2h][plane]
                        for n2h in range(2):
                            row = []
                            for pl in range(2):
                                tb_ = m_in.tile([128, B], PAY, tag="yin_bf")
                                nc.sync.dma_start(
                                    tb_[:], cc1o[g][4 * n2h:4 * (n2h + 1), l, pl, :])
                                t = m_in.tile([128, B], MMD, tag="yin")
                                nc.scalar.copy(t[:], tb_[:])
                                row.append(t)
                            y_t.append(row)

                        # stage B (data as weights): Z^T (n3, k2) in PSUM
                        zt_ps = []  # [n3h][plane]
                        for n3h in range(2):
                            ns = slice(128 * n3h, 128 * (n3h + 1))
                            zr = m_ps.tile([128, B], F32, tag="mps")
                            zi = m_ps.tile([128, B], F32, tag="mps")
                            for n2h in range(2):
                                st = n2h == 0
                                sp = n2h == 1
                                nc.tensor.matmul(zr[:], _mm(y_t[n2h][0][:, ns]), _mm(dr_row[n2h][:]),
                                                 start=st, stop=False, skip_group_check=True)
                                nc.tensor.matmul(zi[:], _mm(y_t[n2h][0][:, ns]), _mm(di_row[n2h][:]),
                                                 start=st, stop=False, skip_group_check=True)
                                nc.tensor.matmul(zr[:], _mm(y_t[n2h][1][:, ns]), _mm(ndi_row[n2h][:]),
                                                 start=False, stop=sp, skip_group_check=True)
                                nc.tensor.matmul(zi[:], _mm(y_t[n2h][1][:, ns]), _mm(dr_row[n2h][:]),
                                                 start=False, stop=sp, skip_group_check=True)
                            zt_ps.append((zr, zi))

                        # T2 twiddle (Z^T layout: mult by T2^T rows) -> SBUF
                        zt_sb = []
                        for n3h in range(2):
                            zr, zi = zt_ps[n3h]
                            or_ = m_sb.tile([128, B], MMD, tag="zt")
                            oi_ = m_sb.tile([128, B], MMD, tag="zt")
                            p1 = m_sb.tile([128, B], F32, tag="mtmp")
                            p2 = m_sb.tile([128, B], F32, tag="mtmp")
                            p3 = m_sb.tile([128, B], F32, tag="mtmp")
                            p4 = m_sb.tile([128, B], F32, tag="mtmp")
                            nc.vector.tensor_mul(p1[:], zr[:], t2r_row[n3h][:])
                            nc.vector.tensor_mul(p2[:], zi[:], t2i_row[n3h][:])
                            nc.vector.tensor_mul(p3[:], zr[:], t2i_row[n3h][:])
                            nc.vector.tensor_mul(p4[:], zi[:], t2r_row[n3h][:])
                            nc.gpsimd.tensor_sub(or_[:], p1[:], p2[:])
                            nc.gpsimd.tensor_add(oi_[:], p3[:], p4[:])
                            zt_sb.append((or_, oi_))

                        # stage C (DFT stationary): U^T (k3, k2) in PSUM
                        ut_ps = []
                        for k3h in range(2):
                            ks = slice(128 * k3h, 128 * (k3h + 1))
                            ur = m_ps.tile([128, B], F32, tag="mps")
                            ui = m_ps.tile([128, B], F32, tag="mps")
                            for n3h in range(2):
                                st = n3h == 0
                                sp = n3h == 1
                                nc.tensor.matmul(ur[:], _mm(dr_row[n3h][:, ks]), _mm(zt_sb[n3h][0][:]),
                                                 start=st, stop=False, skip_group_check=True)
                                nc.tensor.matmul(ui[:], _mm(di_row[n3h][:, ks]), _mm(zt_sb[n3h][0][:]),
                                                 start=st, stop=False, skip_group_check=True)
                                nc.tensor.matmul(ur[:], _mm(ndi_row[n3h][:, ks]), _mm(zt_sb[n3h][1][:]),
                                                 start=False, stop=sp, skip_group_check=True)
                                nc.tensor.matmul(ui[:], _mm(dr_row[n3h][:, ks]), _mm(zt_sb[n3h][1][:]),
                                                 start=False, stop=sp, skip_group_check=True)
                            ut_ps.append((ur, ui))

                        # square: S = U^2 (k3, k2) -> SBUF
                        s_sb = []
                        for k3h in range(2):
                            ur, ui = ut_ps[k3h]
                            sr = m_sb.tile([128, B], MMD, tag="ssb")
                            si = m_sb.tile([128, B], MMD, tag="ssb")
                            uc = m_sb.tile([128, B], F32, tag="mtmp")
                            q1 = m_sb.tile([128, B], F32, tag="mtmp")
                            q2 = m_sb.tile([128, B], F32, tag="mtmp")
                            # only one PSUM operand allowed per vector op: stage ur in SBUF
                            nc.scalar.copy(uc[:], ur[:])
                            nc.vector.tensor_add(q1[:], uc[:], ui[:])
                            nc.vector.tensor_sub(q2[:], uc[:], ui[:])
                            nc.vector.scalar_tensor_tensor(
                                si[:], uc[:], 2.0, ui[:],
                                mybir.AluOpType.mult, mybir.AluOpType.mult)
                            nc.gpsimd.tensor_mul(sr[:], q1[:], q2[:])
                            s_sb.append((sr, si))

                        # stage C' (data as weights): Z2 (k2, n3) in PSUM
                        z2_ps = []
                        for k2h in range(2):
                            ks = slice(128 * k2h, 128 * (k2h + 1))
                            zr = m_ps.tile([128, B], F32, tag="mps")
                            zi = m_ps.tile([128, B], F32, tag="mps")
                            for k3h in range(2):
                                st = k3h == 0
                                sp = k3h == 1
                                nc.tensor.matmul(zr[:], _mm(s_sb[k3h][0][:, ks]), _mm(dr_row[k3h][:]),
                                                 start=st, stop=False, skip_group_check=True)
                                nc.tensor.matmul(zi[:], _mm(s_sb[k3h][0][:, ks]), _mm(ndi_row[k3h][:]),
                                                 start=st, stop=False, skip_group_check=True)
                                nc.tensor.matmul(zr[:], _mm(s_sb[k3h][1][:, ks]), _mm(di_row[k3h][:]),
                                                 start=False, stop=sp, skip_group_check=True)
                                nc.tensor.matmul(zi[:], _mm(s_sb[k3h][1][:, ks]), _mm(dr_row[k3h][:]),
                                                 start=False, stop=sp, skip_group_check=True)
                            z2_ps.append((zr, zi))

                        # conj(T2) twiddle (natural (k2, n3) layout) -> SBUF
                        y2_sb = []
                        for k2h in range(2):
                            zr, zi = z2_ps[k2h]
                            or_ = m_sb.tile([128, B], MMD, tag="y2")
                            oi_ = m_sb.tile([128, B], MMD, tag="y2")
                            p1 = m_sb.tile([128, B], F32, tag="mtmp")
                            p2 = m_sb.tile([128, B], F32, tag="mtmp")
                            p3 = m_sb.tile([128, B], F32, tag="mtmp")
                            p4 = m_sb.tile([128, B], F32, tag="mtmp")
                            nc.vector.tensor_mul(p1[:], zr[:], t2r_row[k2h][:])
                            nc.vector.tensor_mul(p2[:], zi[:], t2i_row[k2h][:])
                            nc.vector.tensor_mul(p3[:], zi[:], t2r_row[k2h][:])
                            nc.vector.tensor_mul(p4[:], zr[:], t2i_row[k2h][:])
                            nc.gpsimd.tensor_add(or_[:], p1[:], p2[:])
                            nc.gpsimd.tensor_sub(oi_[:], p3[:], p4[:])
                            y2_sb.append((or_, oi_))

                        # stage B' (DFT stationary, conj D): Y' (n2, n3) in PSUM
                        for n2h in range(2):
                            ns = slice(128 * n2h, 128 * (n2h + 1))
                            yr = m_ps.tile([128, B], F32, tag="mps")
                            yi = m_ps.tile([128, B], F32, tag="mps")
                            for k2h in range(2):
                                st = k2h == 0
                                sp = k2h == 1
                                nc.tensor.matmul(yr[:], _mm(dr_row[k2h][:, ns]), _mm(y2_sb[k2h][0][:]),
                                                 start=st, stop=False, skip_group_check=True)
                                nc.tensor.matmul(yi[:], _mm(dr_row[k2h][:, ns]), _mm(y2_sb[k2h][1][:]),
                                                 start=st, stop=False, skip_group_check=True)
                                nc.tensor.matmul(yr[:], _mm(di_row[k2h][:, ns]), _mm(y2_sb[k2h][1][:]),
                                                 start=False, stop=sp, skip_group_check=True)
                                nc.tensor.matmul(yi[:], _mm(ndi_row[k2h][:, ns]), _mm(y2_sb[k2h][0][:]),
                                                 start=False, stop=sp, skip_group_check=True)
                            # copy to SBUF (cast to bf16) and store to A2A-2 input
                            for pl, ps in ((0, yr), (1, yi)):
                                o = m_out.tile([128, B], PAY, tag="mout")
                                nc.scalar.copy(o[:], ps[:])
                                nc.sync.dma_start(
                                    cc2i[g][4 * n2h:4 * (n2h + 1), l, pl, :, :], o[:])

                    # piece-g output collective flies while group g+1 computes
                    nc.gpsimd.collective_compute(
                        "AllToAll", mybir.AluOpType.bypass, replica_groups=rg,
                        ins=[cc2i[g].opt()], outs=[cc2o[g].opt()])

                if "M" not in phases:
                    for g in range(G):
                        nc.gpsimd.collective_compute(
                            "AllToAll", mybir.AluOpType.bypass, replica_groups=rg,
                            ins=[cc2i[g].opt()], outs=[cc2o[g].opt()])

                # ============ Phase A': conj(T1), inverse stage A (Im only) ============
                if "F" in phases:
                 with tc.tile_pool(name="f_in", bufs=16) as f_in, \
                     tc.tile_pool(name="f_t1", bufs=8) as f_t1, \
                     tc.tile_pool(name="f_tmp", bufs=16) as f_tmp, \
                     tc.tile_pool(name="f_out", bufs=6) as f_outp, \
                     tc.tile_pool(name="f_ps", bufs=4, space="PSUM") as f_ps:
                    for c in range(NCHUNK):
                        ps_o = f_ps.tile([128, CH], F32, tag="fps")
                        for h in range(2):
                            hs = slice(128 * h, 128 * (h + 1))
                            pr_bf = f_in.tile([128, CH], PAY, tag="pin_bf")
                            pi_bf = f_in.tile([128, CH], PAY, tag="pin_bf")
                            for g in range(G):
                                nc.sync.dma_start(
                                    pr_bf[32 * g:32 * (g + 1), :],
                                    cc2o[g][4 * h:4 * (h + 1), :, 0, 2 * c:2 * (c + 1), :])
                                nc.sync.dma_start(
                                    pi_bf[32 * g:32 * (g + 1), :],
                                    cc2o[g][4 * h:4 * (h + 1), :, 1, 2 * c:2 * (c + 1), :])
                            pr = f_in.tile([128, CH], F32, tag="pin")
                            nc.scalar.copy(pr[:], pr_bf[:])
                            pi = f_in.tile([128, CH], F32, tag="pin")
                            nc.scalar.copy(pi[:], pi_bf[:])
                            t1r_t = f_t1.tile([128, CH], F32, tag="ft1")
                            nc.sync.dma_start(t1r_t[:], t1r_in[hs, c * CH:(c + 1) * CH])
                            t1i_t = f_t1.tile([128, CH], F32, tag="ft1")
                            nc.sync.dma_start(t1i_t[:], t1i_in[hs, c * CH:(c + 1) * CH])

                            # Yf = P * conj(T1)
                            yfr = f_tmp.tile([128, CH], MMD, tag="yf")
                            yfi = f_tmp.tile([128, CH], MMD, tag="yf")
                            p1 = f_tmp.tile([128, CH], F32, tag="ftmp")
                            p2 = f_tmp.tile([128, CH], F32, tag="ftmp")
                            p3 = f_tmp.tile([128, CH], F32, tag="ftmp")
                            p4 = f_tmp.tile([128, CH], F32, tag="ftmp")
                            nc.vector.tensor_mul(p1[:], pr[:], t1r_t[:])
                            nc.gpsimd.tensor_mul(p2[:], pi[:], t1i_t[:])
                            nc.vector.tensor_mul(p3[:], pi[:], t1r_t[:])
                            nc.gpsimd.tensor_mul(p4[:], pr[:], t1i_t[:])
                            nc.vector.tensor_add(yfr[:], p1[:], p2[:])
                            nc.vector.tensor_sub(yfi[:], p3[:], p4[:])

                            st = h == 0
                            sp = h == 1
                            nc.tensor.matmul(ps_o[:], _mm(aw1_blk[h][:]), _mm(yfi[:]),
                                             start=st, stop=False, skip_group_check=True)
                            nc.tensor.matmul(ps_o[:], _mm(aw2_blk[h][:]), _mm(yfr[:]),
                                             start=False, stop=sp, skip_group_check=True)

                        o = f_outp.tile([128, CH], F32, tag="fout")
                        nc.scalar.copy(o[:], ps_o[:])
                        nc.sync.dma_start(y_out[:, c * CH:(c + 1) * CH], o[:])

    nc.compile()
    return nc


_NC = None
_TABLES = None


def _tables():
    global _TABLES
    if _TABLES is None:
        k = np.arange(B)
        D = np.exp(-2j * np.pi * np.outer(k, k) / B)
        T2 = np.exp(-2j * np.pi * np.outer(k, k) / R)
        s = 1.0 / (2.0 * M)
        dr = np.ascontiguousarray(D.real.astype(np.float32))
        di = np.ascontiguousarray(D.imag.astype(np.float32))
        # k1 relabeling: partition p (within an h-half) holds k1loc(p) where
        # p = 32g + 8j + l  ->  k1loc = 32j + 8g + l
        p = np.arange(128)
        k1loc = 32 * ((p % 32) // 8) + 8 * (p // 32) + (p % 8)
        perm = np.concatenate([k1loc, 128 + k1loc])  # rows (h*128+p) -> k1
        t1s = []
        for c in range(W):
            r = np.arange(c * RL, (c + 1) * RL)
            T1 = np.exp(-2j * np.pi * np.outer(k, r) / M)
            T1p = T1[perm, :]
            t1s.append((np.ascontiguousarray(T1p.real.astype(np.float32)),
                        np.ascontiguousarray(T1p.imag.astype(np.float32))))
        pd = D[:128, :][:, perm]
        aw1 = (s * D.real[:, :128])[perm, :]
        aw2 = (-s * D.imag[:, :128])[perm, :]
        _TABLES = dict(
            dr=dr, di=di, ndi=np.ascontiguousarray(-di),
            pdr=np.ascontiguousarray(pd.real.astype(np.float32)),
            pdi=np.ascontiguousarray(pd.imag.astype(np.float32)),
            pndi=np.ascontiguousarray(-pd.imag.astype(np.float32)),
            t2r=np.ascontiguousarray(T2.real.astype(np.float32)),
            t2i=np.ascontiguousarray(T2.imag.astype(np.float32)),
            aw1=np.ascontiguousarray(aw1.astype(np.float32)),
            aw2=np.ascontiguousarray(aw2.astype(np.float32)),
            t1s=t1s,
        )
    return _TABLES


def _in_maps(a, x):
    tb = _tables()
    a3 = a.reshape(128, W, RL)
    x3 = x.reshape(128, W, RL)
    in_maps = []
    for c in range(W):
        in_maps.append(dict(
            a_c=np.ascontiguousarray(a3[:, c, :]),
            x_c=np.ascontiguousarray(x3[:, c, :]),
            t1r=tb["t1s"][c][0], t1i=tb["t1s"][c][1],
            pdr=tb["pdr"], pdi=tb["pdi"], pndi=tb["pndi"],
            dr=tb["dr"], di=tb["di"], ndi=tb["ndi"],
            t2r=tb["t2r"], t2i=tb["t2i"],
            aw1=tb["aw1"], aw2=tb["aw2"],
        ))
    return in_maps


def kernel(a, x, _want_trace=False, **_unused):
    global _NC
    a = np.asarray(a, dtype=np.float32)
    x = np.asarray(x, dtype=np.float32)
    if _NC is None:
        _NC = build_nc()
    res = run_bass_kernel_spmd(_NC, _in_maps(a, x), core_ids=list(range(W)),
                               trace=_want_trace)
    full = np.empty((128, R), dtype=np.float32)
    for c in range(W):
        full[:, c * RL:(c + 1) * RL] = res.results[c]["y_c"]
    out = full.reshape(-1)
    if _want_trace:
        return out, res
    return out


# revision 17
# speedup vs baseline: 84.8583x; 1.0646x over previous
"""FFT-based linear convolution of two 2^23-point real signals on 8 trn2 NeuronCores.

Math: conv(a, x) = Im(ifft(fft(a + i*x)^2)) / 2, with the 2^24-point FFT done as a
3-factor (256^3) matmul FFT. Stage A (over n1) is computed r-sharded across cores,
one AllToAll reshards to k1-sharded for the middle row-FFTs (stages B, C), the
pointwise square happens in the digit-reversed domain, then the inverse stages
(C', B') run locally, a second AllToAll reshards back, and inverse stage A'
produces only the imaginary part of the first half of the time-domain signal.

Both AllToAlls are split into G pieces along the k1 dim so they overlap with the
middle row-FFT loop: middle group g starts once piece g has landed, and the
piece-g output collective flies while group g+1 computes. A k1 relabeling
(host-side permutation of the stage-A stationary columns, T1 rows and inverse
stage-A weight rows) makes each piece a contiguous 32-partition slab so pieces
live in separate DRAM tensors with precise Tile dependencies.

Matmul operands are float32r (4x the fp32 PE rate at ~13-bit mantissa); the
collective payloads ride in bf16 (halves HBM + link traffic) and are cast back
to f32r/f32 right after load. Measured on-device ~0.76 ms/exec vs ~11.5 ms for
the original fp32 serial version; rel err 2.6e-3 vs the 2e-2 gate.
"""
import os
import numpy as np

os.environ.setdefault("JAX_PLATFORMS", "")
import jax

jax.config.update("jax_compilation_cache_dir", "/tmp/jax_neff_cache")
jax.config.update("jax_persistent_cache_min_entry_size_bytes", -1)
jax.config.update("jax_persistent_cache_min_compile_time_secs", 0)

import concourse.bass as bass
import concourse.tile as tile
from concourse import bacc, mybir
from concourse.bass_utils import run_bass_kernel_spmd

N = 8388608          # input length
M = 2 * N            # FFT size = 2^24
B = 256              # radix
R = B * B            # 65536
W = 8                # cores
RL = R // W          # 8192 columns of r per core
CH = 512             # free-dim chunk in stages A / A'
NCHUNK = RL // CH    # 16
G = 4                # A2A pieces (k1l split 32 -> G groups of 32/G)
GL = 32 // G         # k1l values per piece
F32 = mybir.dt.float32

# matmul operands use float32r (~13-bit mantissa, 4x the fp32 PE rate).
USE_F32R = True
MMD = mybir.dt.float32r if USE_F32R else F32
# A2A payloads (cc1/cc2) ride in bf16: halves HBM + link traffic for both
# collectives; loads are cast back to f32r/f32 before any matmul.
PAY = mybir.dt.bfloat16


def _mm(ap):
    return ap


def build_nc(reps=1, phases="AMF"):
    # phases: experiment-only knob ("A"=stage A, "M"=middle, "F"=stage A';
    # collectives always emitted). kernel() always uses the full "AMF".
    nc = bacc.Bacc("TRN2", target_bir_lowering=False, debug=False, num_devices=W)

    a_in = nc.dram_tensor("a_c", [128, RL], MMD, kind="ExternalInput")
    x_in = nc.dram_tensor("x_c", [128, RL], MMD, kind="ExternalInput")
    t1r_in = nc.dram_tensor("t1r", [B, RL], F32, kind="ExternalInput")
    t1i_in = nc.dram_tensor("t1i", [B, RL], F32, kind="ExternalInput")
    pdr_in = nc.dram_tensor("pdr", [128, B], MMD, kind="ExternalInput")
    pdi_in = nc.dram_tensor("pdi", [128, B], MMD, kind="ExternalInput")
    pndi_in = nc.dram_tensor("pndi", [128, B], MMD, kind="ExternalInput")
    dr_in = nc.dram_tensor("dr", [B, B], MMD, kind="ExternalInput")
    di_in = nc.dram_tensor("di", [B, B], MMD, kind="ExternalInput")
    ndi_in = nc.dram_tensor("ndi", [B, B], MMD, kind="ExternalInput")
    t2r_in = nc.dram_tensor("t2r", [B, B], F32, kind="ExternalInput")
    t2i_in = nc.dram_tensor("t2i", [B, B], F32, kind="ExternalInput")
    aw1_in = nc.dram_tensor("aw1", [B, 128], MMD, kind="ExternalInput")
    aw2_in = nc.dram_tensor("aw2", [B, 128], MMD, kind="ExternalInput")
    y_out = nc.dram_tensor("y_c", [128, RL], F32, kind="ExternalOutput")

    rg = [list(range(W))]

    with tile.TileContext(nc) as tc:
        with tc.tile_pool(name="dram", bufs=1, space="DRAM") as dram, \
             tc.tile_pool(name="consts", bufs=1) as consts:
            cc1i = [dram.tile([W, GL, 2, RL], PAY, name=f"cc1i_{g}", tag=f"cc1i_{g}")
                    for g in range(G)]
            cc1o = [dram.tile([W, GL, 2, RL], PAY, name=f"cc1o_{g}", tag=f"cc1o_{g}")
                    for g in range(G)]
            cc2i = [dram.tile([W, GL, 2, 32, B], PAY, name=f"cc2i_{g}", tag=f"cc2i_{g}")
                    for g in range(G)]
            cc2o = [dram.tile([W, GL, 2, 32, B], PAY, name=f"cc2o_{g}", tag=f"cc2o_{g}")
                    for g in range(G)]

            # ---- constant tables in SBUF ----
            # phase-A stationary (cols k1-permuted), [128 n1, 256 k1p]
            pa_st = []
            for src in (pdr_in, pdi_in, pndi_in):
                t = consts.tile([128, B], MMD, name=f"c_{src.name}", tag=f"c_{src.name}")
                nc.sync.dma_start(t[:], src[:, :])
                pa_st.append(t)
            pdr_t, pdi_t, pndi_t = pa_st
            # middle D row-halves (128, 256); col-slices give 128x128 blocks.
            dr_row, di_row, ndi_row, t2r_row, t2i_row = [], [], [], [], []
            for p in range(2):
                for lst, src, dt_ in ((dr_row, dr_in, MMD), (di_row, di_in, MMD), (ndi_row, ndi_in, MMD),
                                 (t2r_row, t2r_in, F32), (t2i_row, t2i_in, F32)):
                    t = consts.tile([128, B], dt_, name=f"c_{src.name}_{p}", tag=f"c_{src.name}_{p}")
                    nc.sync.dma_start(t[:], src[128 * p:128 * (p + 1), :])
                    lst.append(t)
            aw1_blk, aw2_blk = [], []
            for p in range(2):
                for lst, src in ((aw1_blk, aw1_in), (aw2_blk, aw2_in)):
                    t = consts.tile([128, 128], MMD, name=f"c_{src.name}_{p}", tag=f"c_{src.name}_{p}")
                    nc.sync.dma_start(t[:], src[128 * p:128 * (p + 1), :])
                    lst.append(t)

            for _rep in range(reps):
                # ================= Phase A: stage A + T1 twiddle =================
                if "A" in phases:
                 with tc.tile_pool(name="a_io", bufs=1) as a_io, \
                     tc.tile_pool(name="a_t1", bufs=8) as a_t1, \
                     tc.tile_pool(name="a_tmp", bufs=16) as a_tmp, \
                     tc.tile_pool(name="a_out", bufs=6) as a_outp, \
                     tc.tile_pool(name="a_ps", bufs=4, space="PSUM") as a_ps:
                    a_full = a_io.tile([128, RL], MMD)
                    nc.sync.dma_start(a_full[:], a_in[:, :])
                    x_full = a_io.tile([128, RL], MMD)
                    nc.sync.dma_start(x_full[:], x_in[:, :])

                    for c in range(NCHUNK):
                        a_sl = a_full[:, c * CH:(c + 1) * CH]
                        x_sl = x_full[:, c * CH:(c + 1) * CH]
                        for h in range(2):
                            hs = slice(128 * h, 128 * (h + 1))
                            ps_r = a_ps.tile([128, CH], F32, tag="ps")
                            ps_i = a_ps.tile([128, CH], F32, tag="ps")
                            nc.tensor.matmul(ps_r[:], _mm(pdr_t[:, hs]), _mm(a_sl),
                                             start=True, stop=False)
                            nc.tensor.matmul(ps_i[:], _mm(pdr_t[:, hs]), _mm(x_sl),
                                             start=True, stop=False)
                            nc.tensor.matmul(ps_r[:], _mm(pndi_t[:, hs]), _mm(x_sl),
                                             start=False, stop=True)
                            nc.tensor.matmul(ps_i[:], _mm(pdi_t[:, hs]), _mm(a_sl),
                                             start=False, stop=True)

                            t1r_t = a_t1.tile([128, CH], F32, tag="t1")
                            nc.sync.dma_start(t1r_t[:], t1r_in[hs, c * CH:(c + 1) * CH])
                            t1i_t = a_t1.tile([128, CH], F32, tag="t1")
                            nc.sync.dma_start(t1i_t[:], t1i_in[hs, c * CH:(c + 1) * CH])

                            # Y' = (ps_r + i ps_i) * (t1r + i t1i), packed [Re | Im]
                            out_t = a_outp.tile([128, 2 * CH], PAY, tag="aout")
                            m1 = a_tmp.tile([128, CH], F32, tag="tmp")
                            m2 = a_tmp.tile([128, CH], F32, tag="tmp")
                            m3 = a_tmp.tile([128, CH], F32, tag="tmp")
                            m4 = a_tmp.tile([128, CH], F32, tag="tmp")
                            nc.vector.tensor_mul(m1[:], ps_r[:], t1r_t[:])
                            nc.vector.tensor_mul(m2[:], ps_i[:], t1i_t[:])
                            nc.vector.tensor_mul(m3[:], ps_r[:], t1i_t[:])
                            nc.vector.tensor_mul(m4[:], ps_i[:], t1r_t[:])
                            nc.gpsimd.tensor_sub(out_t[:, 0:CH], m1[:], m2[:])
                            nc.gpsimd.tensor_add(out_t[:, CH:2 * CH], m3[:], m4[:])

                            # store piece slabs: partitions [32g,32g+32) hold
                            # k1 = 128h + 32j + 8g + l (j-major, l-minor)
                            for g in range(G):
                                nc.sync.dma_start(
                                    cc1i[g][4 * h:4 * (h + 1), :, :, c * CH:(c + 1) * CH],
                                    out_t[32 * g:32 * (g + 1), :])

                for g in range(G):
                    nc.gpsimd.collective_compute(
                        "AllToAll", mybir.AluOpType.bypass, replica_groups=rg,
                        ins=[cc1i[g].opt()], outs=[cc1o[g].opt()])

                # ================= Middle: per-k1 row FFT + square ================
                if "M" in phases:
                 with tc.tile_pool(name="m_in", bufs=16) as m_in, \
                     tc.tile_pool(name="m_sb", bufs=16) as m_sb, \
                     tc.tile_pool(name="m_out", bufs=12) as m_out, \
                     tc.tile_pool(name="m_ps", bufs=8, space="PSUM") as m_ps:
                  for g in range(G):
                    for l in range(GL):
                        # load Y[k1] as (n2, n3), split in n2-halves, planes;
                        # cast bf16 payload -> f32r for the stage-B stationaries
                        y_t = []  # [n2h][plane]
                        for n2h in range(2):
                            row = []
                            for pl in range(2):
                                tb_ = m_in.tile([128, B], PAY, tag="yin_bf")
                                nc.sync.dma_start(
                                    tb_[:], cc1o[g][4 * n2h:4 * (n2h + 1), l, pl, :])
                                t = m_in.tile([128, B], MMD, tag="yin")
                                nc.scalar.copy(t[:], tb_[:])
                                row.append(t)
                            y_t.append(row)

                        # stage B (data as weights): Z^T (n3, k2) in PSUM
                        zt_ps = []  # [n3h][plane]
                        for n3h in range(2):
                            ns = slice(128 * n3h, 128 * (n3h + 1))
                            zr = m_ps.tile([128, B], F32, tag="mps")
                            zi = m_ps.tile([128, B], F32, tag="mps")
                            for n2h in range(2):
                                st = n2h == 0
                                sp = n2h == 1
                                nc.tensor.matmul(zr[:], _mm(y_t[n2h][0][:, ns]), _mm(dr_row[n2h][:]),
                                                 start=st, stop=False, skip_group_check=True)
                                nc.tensor.matmul(zi[:], _mm(y_t[n2h][0][:, ns]), _mm(di_row[n2h][:]),
                                                 start=st, stop=False, skip_group_check=True)
                                nc.tensor.matmul(zr[:], _mm(y_t[n2h][1][:, ns]), _mm(ndi_row[n2h][:]),
                                                 start=False, stop=sp, skip_group_check=True)
                                nc.tensor.matmul(zi[:], _mm(y_t[n2h][1][:, ns]), _mm(dr_row[n2h][:]),
                                                 start=False, stop=sp, skip_group_check=True)
                            zt_ps.append((zr, zi))

                        # T2 twiddle (Z^T layout: mult by T2^T rows) -> SBUF
                        zt_sb = []
                        for n3h in range(2):
                            zr, zi = zt_ps[n3h]
                            or_ = m_sb.tile([128, B], MMD, tag="zt")
                            oi_ = m_sb.tile([128, B], MMD, tag="zt")
                            p1 = m_sb.tile([128, B], F32, tag="mtmp")
                            p2 = m_sb.tile([128, B], F32, tag="mtmp")
                            p3 = m_sb.tile([128, B], F32, tag="mtmp")
                            p4 = m_sb.tile([128, B], F32, tag="mtmp")
                            nc.vector.tensor_mul(p1[:], zr[:], t2r_row[n3h][:])
                            nc.vector.tensor_mul(p2[:], zi[:], t2i_row[n3h][:])
                            nc.vector.tensor_mul(p3[:], zr[:], t2i_row[n3h][:])
                            nc.vector.tensor_mul(p4[:], zi[:], t2r_row[n3h][:])
                            nc.gpsimd.tensor_sub(or_[:], p1[:], p2[:])
                            nc.gpsimd.tensor_add(oi_[:], p3[:], p4[:])
                            zt_sb.append((or_, oi_))

                        # stage C (DFT stationary): U^T (k3, k2) in PSUM
                        ut_ps = []
                        for k3h in range(2):
                            ks = slice(128 * k3h, 128 * (k3h + 1))
                            ur = m_ps.tile([128, B], F32, tag="mps")
                            ui = m_ps.tile([128, B], F32, tag="mps")
                            for n3h in range(2):
                                st = n3h == 0
                                sp = n3h == 1
                                nc.tensor.matmul(ur[:], _mm(dr_row[n3h][:, ks]), _mm(zt_sb[n3h][0][:]),
                                                 start=st, stop=False, skip_group_check=True)
                                nc.tensor.matmul(ui[:], _mm(di_row[n3h][:, ks]), _mm(zt_sb[n3h][0][:]),
                                                 start=st, stop=False, skip_group_check=True)
                                nc.tensor.matmul(ur[:], _mm(ndi_row[n3h][:, ks]), _mm(zt_sb[n3h][1][:]),
                                                 start=False, stop=sp, skip_group_check=True)
                                nc.tensor.matmul(ui[:], _mm(dr_row[n3h][:, ks]), _mm(zt_sb[n3h][1][:]),
                                                 start=False, stop=sp, skip_group_check=True)
                            ut_ps.append((ur, ui))

                        # square: S = U^2 (k3, k2) -> SBUF
                        s_sb = []
                        for k3h in range(2):
                            ur, ui = ut_ps[k3h]
                            sr = m_sb.tile([128, B], MMD, tag="ssb")
                            si = m_sb.tile([128, B], MMD, tag="ssb")
                            uc = m_sb.tile([128, B], F32, tag="mtmp")
                            q1 = m_sb.tile([128, B], F32, tag="mtmp")
                            q2 = m_sb.tile([128, B], F32, tag="mtmp")
                            # only one PSUM operand allowed per vector op: stage ur in SBUF
                            nc.scalar.copy(uc[:], ur[:])
                            nc.vector.tensor_add(q1[:], uc[:], ui[:])
                            nc.vector.tensor_sub(q2[:], uc[:], ui[:])
                            nc.vector.scalar_tensor_tensor(
                                si[:], uc[:], 2.0, ui[:],
                                mybir.AluOpType.mult, mybir.AluOpType.mult)
                            nc.gpsimd.tensor_mul(sr[:], q1[:], q2[:])
                            s_sb.append((sr, si))

                        # stage C' (data as weights): Z2 (k2, n3) in PSUM
                        z2_ps = []
                        for k2h in range(2):
                            ks = slice(128 * k2h, 128 * (k2h + 1))
                            zr = m_ps.tile([128, B], F32, tag="mps")
                            zi = m_ps.tile([128, B], F32, tag="mps")
                            for k3h in range(2):
                                st = k3h == 0
                                sp = k3h == 1
                                nc.tensor.matmul(zr[:], _mm(s_sb[k3h][0][:, ks]), _mm(dr_row[k3h][:]),
                                                 start=st, stop=False, skip_group_check=True)
                                nc.tensor.matmul(zi[:], _mm(s_sb[k3h][0][:, ks]), _mm(ndi_row[k3h][:]),
                                                 start=st, stop=False, skip_group_check=True)
                                nc.tensor.matmul(zr[:], _mm(s_sb[k3h][1][:, ks]), _mm(di_row[k3h][:]),
                                                 start=False, stop=sp, skip_group_check=True)
                                nc.tensor.matmul(zi[:], _mm(s_sb[k3h][1][:, ks]), _mm(dr_row[k3h][:]),
                                                 start=False, stop=sp, skip_group_check=True)
                            z2_ps.append((zr, zi))

                        # conj(T2) twiddle (natural (k2, n3) layout) -> SBUF
                        y2_sb = []
                        for k2h in range(2):
                            zr, zi = z2_ps[k2h]
                            or_ = m_sb.tile([128, B], MMD, tag="y2")
                            oi_ = m_sb.tile([128, B], MMD, tag="y2")
                            p1 = m_sb.tile([128, B], F32, tag="mtmp")
                            p2 = m_sb.tile([128, B], F32, tag="mtmp")
                            p3 = m_sb.tile([128, B], F32, tag="mtmp")
                            p4 = m_sb.tile([128, B], F32, tag="mtmp")
                            nc.vector.tensor_mul(p1[:], zr[:], t2r_row[k2h][:])
                            nc.vector.tensor_mul(p2[:], zi[:], t2i_row[k2h][:])
                            nc.vector.tensor_mul(p3[:], zi[:], t2r_row[k2h][:])
                            nc.vector.tensor_mul(p4[:], zr[:], t2i_row[k2h][:])
                            nc.gpsimd.tensor_add(or_[:], p1[:], p2[:])
                            nc.gpsimd.tensor_sub(oi_[:], p3[:], p4[:])
                            y2_sb.append((or_, oi_))

                        # stage B' (DFT stationary, conj D): Y' (n2, n3) in PSUM
                        for n2h in range(2):
                            ns = slice(128 * n2h, 128 * (n2h + 1))
                            yr = m_ps.tile([128, B], F32, tag="mps")
                            yi = m_ps.tile([128, B], F32, tag="mps")
                            for k2h in range(2):
                                st = k2h == 0
                                sp = k2h == 1
                                nc.tensor.matmul(yr[:], _mm(dr_row[k2h][:, ns]), _mm(y2_sb[k2h][0][:]),
                                                 start=st, stop=False, skip_group_check=True)
                                nc.tensor.matmul(yi[:], _mm(dr_row[k2h][:, ns]), _mm(y2_sb[k2h][1][:]),
                                                 start=st, stop=False, skip_group_check=True)
                                nc.tensor.matmul(yr[:], _mm(di_row[k2h][:, ns]), _mm(y2_sb[k2h][1][:]),
                                                 start=False, stop=sp, skip_group_check=True)
                                nc.tensor.matmul(yi[:], _mm(ndi_row[k2h][:, ns]), _mm(y2_sb[k2h][0][:]),
                                                 start=False, stop=sp, skip_group_check=True)
                            # copy to SBUF (cast to bf16) and store to A2A-2 input
                            for pl, ps in ((0, yr), (1, yi)):
                                o = m_out.tile([128, B], PAY, tag="mout")
                                nc.scalar.copy(o[:], ps[:])
                                nc.sync.dma_start(
                                    cc2i[g][4 * n2h:4 * (n2h + 1), l, pl, :, :], o[:])

                    # piece-g output collective flies while group g+1 computes
                    nc.gpsimd.collective_compute(
                        "AllToAll", mybir.AluOpType.bypass, replica_groups=rg,
                        ins=[cc2i[g].opt()], outs=[cc2o[g].opt()])

                if "M" not in phases:
                    for g in range(G):
                        nc.gpsimd.collective_compute(
                            "AllToAll", mybir.AluOpType.bypass, replica_groups=rg,
                            ins=[cc2i[g].opt()], outs=[cc2o[g].opt()])

                # ============ Phase A': conj(T1), inverse stage A (Im only) ============
                if "F" in phases:
                 with tc.tile_pool(name="f_in", bufs=16) as f_in, \
                     tc.tile_pool(name="f_t1", bufs=8) as f_t1, \
                     tc.tile_pool(name="f_tmp", bufs=16) as f_tmp, \
                     tc.tile_pool(name="f_out", bufs=6) as f_outp, \
                     tc.tile_pool(name="f_ps", bufs=4, space="PSUM") as f_ps:
                    for c in range(NCHUNK):
                        ps_o = f_ps.tile([128, CH], F32, tag="fps")
                        for h in range(2):
                            hs = slice(128 * h, 128 * (h + 1))
                            pr_bf = f_in.tile([128, CH], PAY, tag="pin_bf")
                            pi_bf = f_in.tile([128, CH], PAY, tag="pin_bf")
                            for g in range(G):
                                nc.sync.dma_start(
                                    pr_bf[32 * g:32 * (g + 1), :],
                                    cc2o[g][4 * h:4 * (h + 1), :, 0, 2 * c:2 * (c + 1), :])
                                nc.sync.dma_start(
                                    pi_bf[32 * g:32 * (g + 1), :],
                                    cc2o[g][4 * h:4 * (h + 1), :, 1, 2 * c:2 * (c + 1), :])
                            pr = f_in.tile([128, CH], F32, tag="pin")
                            nc.scalar.copy(pr[:], pr_bf[:])
                            pi = f_in.tile([128, CH], F32, tag="pin")
                            nc.scalar.copy(pi[:], pi_bf[:])
                            t1r_t = f_t1.tile([128, CH], F32, tag="ft1")
                            nc.sync.dma_start(t1r_t[:], t1r_in[hs, c * CH:(c + 1) * CH])
                            t1i_t = f_t1.tile([128, CH], F32, tag="ft1")
                            nc.sync.dma_start(t1i_t[:], t1i_in[hs, c * CH:(c + 1) * CH])

                            # Yf = P * conj(T1)
                            yfr = f_tmp.tile([128, CH], MMD, tag="yf")
                            yfi = f_tmp.tile([128, CH], MMD, tag="yf")
                            p1 = f_tmp.tile([128, CH], F32, tag="ftmp")
                            p2 = f_tmp.tile([128, CH], F32, tag="ftmp")
                            p3 = f_tmp.tile([128, CH], F32, tag="ftmp")
                            p4 = f_tmp.tile([128, CH], F32, tag="ftmp")
                            nc.vector.tensor_mul(p1[:], pr[:], t1r_t[:])
                            nc.gpsimd.tensor_mul(p2[:], pi[:], t1i_t[:])
                            nc.vector.tensor_mul(p3[:], pi[:], t1r_t[:])
                            nc.gpsimd.tensor_mul(p4[:], pr[:], t1i_t[:])
                            nc.vector.tensor_add(yfr[:], p1[:], p2[:])
                            nc.vector.tensor_sub(yfi[:], p3[:], p4[:])

                            st = h == 0
                            sp = h == 1
                            nc.tensor.matmul(ps_o[:], _mm(aw1_blk[h][:]), _mm(yfi[:]),
                                             start=st, stop=False, skip_group_check=True)
                            nc.tensor.matmul(ps_o[:], _mm(aw2_blk[h][:]), _mm(yfr[:]),
                                             start=False, stop=sp, skip_group_check=True)

                        o = f_outp.tile([128, CH], F32, tag="fout")
                        nc.scalar.copy(o[:], ps_o[:])
                        nc.sync.dma_start(y_out[:, c * CH:(c + 1) * CH], o[:])

    nc.compile()
    return nc


_NC = None
_TABLES = None


def _tables():
    global _TABLES
    if _TABLES is None:
        k = np.arange(B)
        D = np.exp(-2j * np.pi * np.outer(k, k) / B)
        T2 = np.exp(-2j * np.pi * np.outer(k, k) / R)
        s = 1.0 / (2.0 * M)
        dr = np.ascontiguousarray(D.real.astype(np.float32))
        di = np.ascontiguousarray(D.imag.astype(np.float32))
        # k1 relabeling: partition p (within an h-half) holds k1loc(p) where
        # p = 32g + 8j + l  ->  k1loc = 32j + 8g + l
        p = np.arange(128)
        k1loc = 32 * ((p % 32) // 8) + 8 * (p // 32) + (p % 8)
        perm = np.concatenate([k1loc, 128 + k1loc])  # rows (h*128+p) -> k1
        t1s = []
        for c in range(W):
            r = np.arange(c * RL, (c + 1) * RL)
            T1 = np.exp(-2j * np.pi * np.outer(k, r) / M)
            T1p = T1[perm, :]
            t1s.append((np.ascontiguousarray(T1p.real.astype(np.float32)),
                        np.ascontiguousarray(T1p.imag.astype(np.float32))))
        pd = D[:128, :][:, perm]
        aw1 = (s * D.real[:, :128])[perm, :]
        aw2 = (-s * D.imag[:, :128])[perm, :]
        _TABLES = dict(
            dr=dr, di=di, ndi=np.ascontiguousarray(-di),
            pdr=np.ascontiguousarray(pd.real.astype(np.float32)),
            pdi=np.ascontiguousarray(pd.imag.astype(np.float32)),
            pndi=np.ascontiguousarray(-pd.imag.astype(np.float32)),
            t2r=np.ascontiguousarray(T2.real.astype(np.float32)),
            t2i=np.ascontiguousarray(T2.imag.astype(np.float32)),
            aw1=np.ascontiguousarray(aw1.astype(np.float32)),
            aw2=np.ascontiguousarray(aw2.astype(np.float32)),
            t1s=t1s,
        )
    return _TABLES


def _in_maps(a, x):
    tb = _tables()
    a3 = a.reshape(128, W, RL)
    x3 = x.reshape(128, W, RL)
    in_maps = []
    for c in range(W):
        in_maps.append(dict(
            a_c=np.ascontiguousarray(a3[:, c, :]),
            x_c=np.ascontiguousarray(x3[:, c, :]),
            t1r=tb["t1s"][c][0], t1i=tb["t1s"][c][1],
            pdr=tb["pdr"], pdi=tb["pdi"], pndi=tb["pndi"],
            dr=tb["dr"], di=tb["di"], ndi=tb["ndi"],
            t2r=tb["t2r"], t2i=tb["t2i"],
            aw1=tb["aw1"], aw2=tb["aw2"],
        ))
    return in_maps


def kernel(a, x, _want_trace=False, **_unused):
    global _NC
    a = np.asarray(a, dtype=np.float32)
    x = np.asarray(x, dtype=np.float32)
    if _NC is None:
        _NC = build_nc()
    res = run_bass_kernel_spmd(_NC, _in_maps(a, x), core_ids=list(range(W)),
                               trace=_want_trace)
    full = np.empty((128, R), dtype=np.float32)
    for c in range(W):
        full[:, c * RL:(c + 1) * RL] = res.results[c]["y_c"]
    out = full.reshape(-1)
    if _want_trace:
        return out, res
    return out
